# revision 7
# baseline (speedup 1.0000x reference)
"""Trainium2 Bass kernel for the GRU+MLP+fc+out model.

Strategy (8 NeuronCores, data-parallel over batch):
- Each core runs B/8 = 128 batch rows with hidden-on-partitions [H, B] layout.
- x is host-transposed/cast to bf16 [IN, T*Bc] per core; DMA'd in 32-step chunks.
- Two phase-offset half-batch chains (64 cols each) per core fill each other's
  dependency-stall gaps.
- Critical-cycle surgery vs the naive step: h_t = g_t - u_t with
  g = z*a (late, post-tanh) and u = (z-1)*h_{t-1} (early, post-sigmoid).
  The next step's pre-activations accumulate wh*g and (-wh)*u directly in
  PSUM (negated weight copies for the u term), so the h-combine (hn) leaves
  the serial cycle: ... tanh -> g -> wh*g -> sigma(r) -> rh -> wha -> tanh ...
  hn is still formed off-path for the next rh/u and the head matmul.
- sigma is split: sigma_r [H,64] is on the path (waits only whr*g), sigma_z
  follows off-path.
- Head folding (host, f32): P_t = mlp_w @ fc_w_t @ out_w, so
  out = sum_t ys_t @ P_t + d. Exact up to f32 rounding.
"""
import numpy as np
import ml_dtypes

import concourse.bacc as bacc
import concourse.bass as bass
import concourse.mybir as mybir
import concourse.tile as tile
from concourse.bass_utils import run_bass_kernel_spmd

bf16 = ml_dtypes.bfloat16
f32 = np.float32

B, T, IN, H, HOR = 1024, 256, 128, 128, 24
NCORES = 8
BC = B // NCORES  # 128 batch rows per core
CH = 32           # timesteps per x chunk
AF = mybir.ActivationFunctionType
ALU = mybir.AluOpType
DT = mybir.dt

_cache: dict = {}


def _build_module(t_steps: int = T, head_every: int = 8):
    nc = bacc.Bacc("TRN2", target_bir_lowering=False, debug=False)

    xt = nc.dram_tensor("xt", [IN, t_steps * BC], DT.bfloat16, kind="ExternalInput")
    wpack = nc.dram_tensor("wpack", [128, 8 * H], DT.bfloat16, kind="ExternalInput")
    bias3 = nc.dram_tensor("bias3", [H, 3], DT.float32, kind="ExternalInput")
    pmat = nc.dram_tensor("pmat", [H, t_steps * HOR], DT.bfloat16, kind="ExternalInput")
    dvec = nc.dram_tensor("dvec", [HOR, 1], DT.float32, kind="ExternalInput")
    outT = nc.dram_tensor("outT", [HOR, BC], DT.float32, kind="ExternalOutput")

    nchunks = (t_steps + CH - 1) // CH
    HB = BC // 2  # 64 columns per chain

    with tile.TileContext(nc) as tc:
        with (
            tc.tile_pool(name="const", bufs=1) as cpool,
            tc.tile_pool(name="xchunks", bufs=3) as xpool,
            tc.tile_pool(name="state", bufs=max(3, head_every + 2)) as hpool,
            tc.tile_pool(name="work", bufs=3) as wkpool,
            tc.tile_pool(name="pzA", bufs=2, space="PSUM") as zrpoolA,
            tc.tile_pool(name="pzB", bufs=2, space="PSUM") as zrpoolB,
            tc.tile_pool(name="pa", bufs=2, space="PSUM") as apool,
            tc.tile_pool(name="po", bufs=1, space="PSUM") as opool,
        ):
            wt = cpool.tile([128, 8 * H], DT.bfloat16, name="wt")
            nc.sync.dma_start(wt[:, :], wpack.ap())
            bt = cpool.tile([H, 3], DT.float32, name="bt")
            nc.sync.dma_start(bt[:, :], bias3.ap())
            pt = cpool.tile([H, t_steps * HOR], DT.bfloat16, name="pt")
            nc.sync.dma_start(pt[:, :], pmat.ap())
            dt_ = cpool.tile([HOR, 1], DT.float32, name="dt_")
            nc.sync.dma_start(dt_[:, :], dvec.ap())

            wiz, wir, wia = wt[:, 0:H], wt[:, H:2*H], wt[:, 2*H:3*H]
            whz, whr, wha = wt[:, 3*H:4*H], wt[:, 4*H:5*H], wt[:, 5*H:6*H]
            whzN, whrN = wt[:, 6*H:7*H], wt[:, 7*H:8*H]
            bz, br, ba = bt[:, 0:1], bt[:, 1:2], bt[:, 2:3]

            po = opool.tile([HOR, BC], DT.float32, name="po")

            xcs: list = [None] * nchunks

            def load_chunk(c):
                n = min(CH, t_steps - c * CH)
                xc = xpool.tile([IN, CH * BC], DT.bfloat16, tag="xc", name=f"xc{c}")
                nc.sync.dma_start(xc[:, : n * BC], xt.ap()[:, c * CH * BC:(c * CH + n) * BC])
                xcs[c] = xc

            load_chunk(0)
            if nchunks > 1:
                load_chunk(1)

            def xslice(t, j):
                c, off = divmod(t, CH)
                return xcs[c][:, off * BC + j * HB: off * BC + (j + 1) * HB]

            zrpool = [zrpoolA, zrpoolB]
            # per-chain rolling state
            hp = [None, None]     # h_{t-1} tile (bf16 SBUF)
            pzr = [None, None]    # psum [z|r] being read at step t
            pa = [None, None]     # psum a-preact slice being read at step t
            pzr_n = [None, None]  # psum for step t+1 (accumulating)
            pa_n = [None, None]

            def emit_gx(t, final=False):
                """pre-activation gx matmuls for step t, both chains (off-path).
                pa for both chains shares one PSUM tile (bank budget)."""
                q = apool.tile([128, 2 * HB], DT.float32, tag="pa", name=f"pa_{t}")
                for j in (0, 1):
                    p = zrpool[j].tile([128, 2 * HB], DT.float32, tag="pzr",
                                       name=f"pzr{j}_{t}")
                    xs = xslice(t, j)
                    nc.tensor.matmul(p[:, HB:2*HB], wir, xs, start=True, stop=final)
                    nc.tensor.matmul(p[:, 0:HB], wiz, xs, start=False, stop=final)
                    nc.tensor.matmul(q[:, j*HB:(j+1)*HB], wia, xs, start=(j == 0),
                                     stop=final)
                    pzr_n[j] = p
                    pa_n[j] = q[:, j*HB:(j+1)*HB]

            pending_heads: list = []

            def flush_heads():
                for (ht, hj, hn, first, last) in pending_heads:
                    nc.tensor.matmul(po[:, hj*HB:(hj+1)*HB],
                                     pt[:, ht*HOR:(ht+1)*HOR], hn[:, :],
                                     start=first, stop=last)
                pending_heads.clear()

            # ---- t = 0: h = 0 -> r/u drop out; h1 = sigmoid(gx_z)*tanh(gx_a)
            z0 = [None, None]
            a0 = [None, None]
            emit_gx(0, final=True)
            for j in (0, 1):
                pzr[j], pa[j] = pzr_n[j], pa_n[j]
            for j in (0, 1):
                z = wkpool.tile([H, HB], DT.bfloat16, tag=f"z{j}", name=f"z0_{j}")
                nc.scalar.activation(z[:, :], pzr[j][:, 0:HB], AF.Sigmoid, bias=bz)
                z0[j] = z
            for j in (0, 1):
                a = wkpool.tile([H, HB], DT.bfloat16, tag=f"a{j}", name=f"a0_{j}")
                nc.scalar.activation(a[:, :], pa[j][:, :], AF.Tanh, bias=ba)
                a0[j] = a
            for j in (0, 1):
                # g0 = z0*a0 == h_1; doubles as the head operand for t=0
                hn = hpool.tile([H, HB], DT.bfloat16, tag=f"h{j}", name=f"h{j}_1")
                nc.vector.tensor_mul(hn[:, :], z0[j][:, :], a0[j][:, :])
                nc.tensor.matmul(po[:, j*HB:(j+1)*HB], pt[:, 0:HOR], hn[:, :],
                                 start=(j == 0), stop=(t_steps == 1 and j == 1))
                hp[j] = hn
            if t_steps > 1:
                emit_gx(1)
                for j in (0, 1):
                    # close both psum regions: only g feeds step 1 (u_0 = 0)
                    nc.tensor.matmul(pzr_n[j][:, HB:2*HB], whr, hp[j][:, :],
                                     start=False, stop=True)
                    nc.tensor.matmul(pzr_n[j][:, 0:HB], whz, hp[j][:, :],
                                     start=False, stop=True)
                for j in (0, 1):
                    pzr[j], pa[j] = pzr_n[j], pa_n[j]

            for t in range(1, t_steps):
                c, off = divmod(t, CH)
                if off == 0 and c + 1 < nchunks:
                    load_chunk(c + 1)
                last_step = t == t_steps - 1

                # stage 1: prefetch gx(t+1) [PE, ready as soon as x chunk is in]
                if not last_step:
                    emit_gx(t + 1)
                # stage 2: sigma_r on the path, sigma_z trailing
                rt = [None, None]
                zt = [None, None]
                for j in (0, 1):
                    r = wkpool.tile([H, HB], DT.bfloat16, tag=f"r{j}", name=f"r{j}_{t}")
                    nc.scalar.activation(r[:, :], pzr[j][:, HB:2*HB], AF.Sigmoid,
                                         bias=br)
                    rt[j] = r
                for j in (0, 1):
                    z = wkpool.tile([H, HB], DT.bfloat16, tag=f"z{j}", name=f"z{j}_{t}")
                    nc.scalar.activation(z[:, :], pzr[j][:, 0:HB], AF.Sigmoid, bias=bz)
                    zt[j] = z
                # stage 3: rh on the path, u = (z-1)*h trailing
                rh = [None, None]
                ut = [None, None]
                for j in (0, 1):
                    r_h = wkpool.tile([H, HB], DT.bfloat16, tag=f"rh{j}",
                                      name=f"rh{j}_{t}")
                    nc.vector.tensor_mul(r_h[:, :], rt[j][:, :], hp[j][:, :])
                    rh[j] = r_h
                for j in (0, 1):
                    u = wkpool.tile([H, HB], DT.bfloat16, tag=f"u{j}", name=f"u{j}_{t}")
                    nc.vector.scalar_tensor_tensor(u[:, :], zt[j][:, :], 1.0,
                                                   hp[j][:, :],
                                                   op0=ALU.subtract, op1=ALU.mult)
                    ut[j] = u
                # stage 4: wha*rh (path), then -wh*u accumulation for t+1
                for j in (0, 1):
                    nc.tensor.matmul(pa[j][:, :], wha, rh[j][:, :],
                                     start=False, stop=True)
                if not last_step:
                    for j in (0, 1):
                        nc.tensor.matmul(pzr_n[j][:, HB:2*HB], whrN, ut[j][:, :],
                                         start=False, stop=False)
                        nc.tensor.matmul(pzr_n[j][:, 0:HB], whzN, ut[j][:, :],
                                         start=False, stop=False)
                # stage 5: tanh
                at = [None, None]
                for j in (0, 1):
                    a = wkpool.tile([H, HB], DT.bfloat16, tag=f"a{j}", name=f"a{j}_{t}")
                    nc.scalar.activation(a[:, :], pa[j][:, :], AF.Tanh, bias=ba)
                    at[j] = a
                # stage 6: g = z*a (path), hn = g - u (off-path)
                gt = [None, None]
                for j in (0, 1):
                    g = wkpool.tile([H, HB], DT.bfloat16, tag=f"g{j}", name=f"g{j}_{t}")
                    nc.vector.tensor_mul(g[:, :], zt[j][:, :], at[j][:, :])
                    gt[j] = g
                for j in (0, 1):
                    hn = hpool.tile([H, HB], DT.bfloat16, tag=f"h{j}",
                                    name=f"h{j}_{t+1}")
                    nc.vector.tensor_sub(hn[:, :], gt[j][:, :], ut[j][:, :])
                    pending_heads.append((t, j, hn, False, last_step and j == 1))
                    hp[j] = hn
                # stage 7: wh*g closes the t+1 psum regions (whr*g is the
                # path edge into sigma_r(t+1)); heads flushed after
                if not last_step:
                    for j in (0, 1):
                        nc.tensor.matmul(pzr_n[j][:, HB:2*HB], whr, gt[j][:, :],
                                         start=False, stop=True)
                        nc.tensor.matmul(pzr_n[j][:, 0:HB], whz, gt[j][:, :],
                                         start=False, stop=True)
                if t % head_every == head_every - 1 or last_step:
                    flush_heads()
                if not last_step:
                    for j in (0, 1):
                        pzr[j], pa[j] = pzr_n[j], pa_n[j]

            osb = cpool.tile([HOR, BC], DT.float32, name="osb")
            nc.scalar.add(osb[:, :], po[:, :], dt_[:, 0:1])
            nc.sync.dma_start(outT.ap(), osb[:, :])

    nc.compile()
    return nc


def _get_module(t_steps: int = T):
    key = ("nc", t_steps)
    if key not in _cache:
        _cache[key] = _build_module(t_steps)
    return _cache[key]


def _prep_inputs(x, w_i, w_h, b, mlp_w, mlp_b, fc_w, fc_b, out_w, out_b):
    x = np.asarray(x, f32)
    w_i = np.asarray(w_i, f32); w_h = np.asarray(w_h, f32); b = np.asarray(b, f32)
    mlp_w = np.asarray(mlp_w, f32); mlp_b = np.asarray(mlp_b, f32)
    fc_w = np.asarray(fc_w, f32); fc_b = np.asarray(fc_b, f32)
    out_w = np.asarray(out_w, f32); out_b = np.asarray(out_b, f32)

    # folded head: P_t = mlp_w @ fc_w_t @ out_w ; d = (mlp_b @ sum_t fc_w_t + fc_b) @ out_w + out_b
    W2 = fc_w @ out_w                                     # [T*4H, HOR]
    P = mlp_w @ W2.reshape(T, 4 * H, HOR).transpose(1, 0, 2).reshape(4 * H, T * HOR)
    Pm = np.ascontiguousarray(P.astype(bf16))             # [H, T*HOR]
    d = (mlp_b @ fc_w.reshape(T, 4 * H, H).sum(0) + fc_b) @ out_w + out_b

    w_h_neg = -w_h[:, :2*H]  # [whzN | whrN]
    wpack = np.ascontiguousarray(
        np.concatenate([w_i, w_h, w_h_neg], axis=1).astype(bf16))
    bias3 = np.ascontiguousarray(
        np.stack([b[:H], b[H:2*H], b[2*H:]], axis=1).astype(f32))
    dvec = np.ascontiguousarray(d.reshape(HOR, 1).astype(f32))

    xbf = x.astype(bf16)
    shared = {"wpack": wpack, "bias3": bias3, "pmat": Pm, "dvec": dvec}
    in_maps = []
    for c in range(NCORES):
        xt_c = np.ascontiguousarray(
            xbf[c*BC:(c+1)*BC].transpose(2, 1, 0).reshape(IN, T * BC))
        in_maps.append({"xt": xt_c, **shared})
    return in_maps


def run(inputs: dict, trace: bool = False, **kw):
    nc = _get_module(T)
    in_maps = _prep_inputs(**inputs)
    res = run_bass_kernel_spmd(nc, in_maps, core_ids=list(range(NCORES)),
                               trace=trace, **kw)
    out = np.empty((B, HOR), f32)
    for c in range(NCORES):
        out[c*BC:(c+1)*BC, :] = res.results[c]["outT"].T
    return out, res


def kernel(**inputs) -> np.ndarray:
    out, _ = run(inputs)
    return out
